# revision 12
# baseline (speedup 1.0000x reference)
"""Trainium2 Bass kernel for nn_DAttention:
out[b,c,d,h,w] = x[b,c,d,h,w] * mean_{c,h,w}(x[b,:,d,:,:]).

Sharding: pure data parallel over batch B=8 -> one batch per NeuronCore
(x[b] is a contiguous zero-copy slice). Numerics: HBM I/O in bf16 (the
host converts f32->bf16 on the way in and bf16->f32 on the way out);
the mean is accumulated in fp32 (ACT accum_out + fp32 matmuls), so the
element error is ~2 bf16 roundings (~0.4%) -- far inside the 2e-2
gate. This halves HBM traffic vs f32: 32 MiB in + 32 MiB out per core,
the memory-roofline minimum for this regime.

Per core, loop over the 32 d-slices (1 MiB each in bf16): load
x[b,:,d,:,:] into SBUF, reduce to the scalar mean, multiply, store.

SBUF layout per d-slice: tile [128, 4096] bf16 with partition
p = c*4 + hg (H split into 4 groups of 32), free = (h%32)*128 + w.
Each partition row is one contiguous 8192-byte DRAM run -> every DMA
descriptor is a power-of-2 8 KiB (measured: 8/16 KiB descriptors run
at line rate +8 ns; odd sizes like 6.3/7.1 KiB pay +30-80 ns each, and
non-multiple-of-8 row counts fall off the 16-engine descriptor
balancer entirely).

Engine schedule per slice (balanced so neither ACT nor DVE paces the
post-load tail far above the store-DMA rate):
  ACT: activation-Copy of xt[:, :A] into a dead PSUM scratch with
       accum_out -> fp32 column sums (the PSUM write is free bandwidth;
       only Matmul/Memset may write bf16 to PSUM, so scratch is f32)
  DVE: tensor_reduce(add) of xt[:, A:] -> fp32 column sums
  PE : two accumulated fp32 matmuls against a constant 128x128 matrix
       of 1/2^19 -> cross-partition sum + broadcast of the mean to all
       partitions in one PSUM [128,1]
  ACT: tiny copy of the mean PSUM->SBUF
  DVE: single tensor_scalar multiply (bf16, ~4 elem/cyc/lane)
  DMA: loads on the SP HWDGE ring, stores on the ACT ring
"""
import numpy as np
import ml_dtypes

import concourse.bacc as bacc
import concourse.tile as tile
import concourse.mybir as mybir
from concourse.bass_utils import run_bass_kernel_spmd

BF16 = ml_dtypes.bfloat16

B, C, D, H, W = 8, 32, 32, 128, 128
HG, HL = 4, 32          # H split: partition dim = C*HG = 128
P = C * HG              # 128 partitions
F = HL * W              # 4096 free elements per partition
N_RED = C * H * W       # 524288 = 2**19 elements reduced per (b, d)
RECIP = 1.0 / N_RED     # exact in fp32
A_SPLIT = 2432          # ACT reduces xt[:, :A], DVE reduces xt[:, A:]

_NC = None


def _build_nc(xin_bufs=7, out_bufs=3):
    nc = bacc.Bacc("TRN2", target_bir_lowering=False, debug=False)
    x5 = nc.dram_tensor("x", [C, D, HG, HL, W], mybir.dt.bfloat16, kind="ExternalInput")
    o5 = nc.dram_tensor("out", [C, D, HG, HL, W], mybir.dt.bfloat16, kind="ExternalOutput")
    with tile.TileContext(nc) as tc:
        with (
            tc.tile_pool(name="xin", bufs=xin_bufs) as xpool,
            tc.tile_pool(name="oout", bufs=out_bufs) as opool,
            tc.tile_pool(name="small", bufs=6) as spool,
            tc.tile_pool(name="psum", bufs=2, space="PSUM") as ppool,
            tc.tile_pool(name="psc", bufs=1, space="PSUM") as scpool,
            tc.tile_pool(name="const", bufs=1) as cpool,
        ):
            recip = cpool.tile([P, P], mybir.dt.float32)
            nc.gpsimd.memset(recip[:], RECIP)
            for d in range(D):
                xt = xpool.tile([P, F], mybir.dt.bfloat16, tag="xt")
                nc.sync.dma_start(xt[:], x5[:, d])
                csa = spool.tile([P, 1], mybir.dt.float32, tag="csa")
                csb = spool.tile([P, 1], mybir.dt.float32, tag="csb")
                scratch = scpool.tile([P, A_SPLIT], mybir.dt.float32, tag="sc")
                nc.scalar.activation(
                    scratch[:], xt[:, :A_SPLIT],
                    mybir.ActivationFunctionType.Copy, accum_out=csa[:],
                )
                nc.vector.tensor_reduce(
                    csb[:], xt[:, A_SPLIT:],
                    mybir.AxisListType.X, mybir.AluOpType.add,
                )
                dv = ppool.tile([P, 1], mybir.dt.float32, tag="dv")
                nc.tensor.matmul(dv[:], recip[:], csa[:], start=True, stop=False)
                nc.tensor.matmul(dv[:], recip[:], csb[:], start=False, stop=True)
                dvs = spool.tile([P, 1], mybir.dt.float32, tag="dvs")
                nc.scalar.copy(dvs[:], dv[:])
                ot = opool.tile([P, F], mybir.dt.bfloat16, tag="ot")
                nc.vector.tensor_scalar_mul(ot[:], xt[:], dvs[:])
                nc.scalar.dma_start(o5[:, d], ot[:])
    nc.compile()
    return nc


def _get_nc():
    global _NC
    if _NC is None:
        _NC = _build_nc()
    return _NC


def run(x: np.ndarray, trace: bool = False, tmpdir: str | None = None):
    """Run on 8 NeuronCores; returns (out, BassKernelResults)."""
    x = np.asarray(x)
    assert x.shape == (B, C, D, H, W), x.shape
    xb = x.astype(BF16)
    nc = _get_nc()
    in_maps = [
        {"x": np.ascontiguousarray(xb[b]).reshape(C, D, HG, HL, W)} for b in range(B)
    ]
    res = run_bass_kernel_spmd(
        nc, in_maps, core_ids=list(range(B)), trace=trace, tmpdir=tmpdir
    )
    out = np.stack(
        [r["out"].astype(np.float32).reshape(C, D, H, W) for r in res.results]
    )
    return out, res


def kernel(x: np.ndarray) -> np.ndarray:
    out, _ = run(x)
    return out


# revision 13
# speedup vs baseline: 1.5514x; 1.5514x over previous
"""Trainium2 Bass kernel for nn_DAttention:
out[b,c,d,h,w] = x[b,c,d,h,w] * mean_{c,h,w}(x[b,:,d,:,:]).

Sharding: pure data parallel over batch B=8 -> one batch per NeuronCore
(x[b] is a contiguous zero-copy slice). Numerics: HBM I/O in bf16 (the
host converts f32->bf16 on the way in and bf16->f32 on the way out);
the mean is accumulated in fp32 (ACT accum_out + fp32 matmuls), so the
element error is ~2 bf16 roundings (~0.4%) -- far inside the 2e-2
gate. This halves HBM traffic vs f32: 32 MiB in + 32 MiB out per core,
the memory-roofline minimum for this regime.

Per core, loop over the 32 d-slices (1 MiB each in bf16): load
x[b,:,d,:,:] into SBUF, reduce to the scalar mean, multiply, store.

SBUF layout per d-slice: tile [128, 4096] bf16 with partition
p = c*4 + hg (H split into 4 groups of 32), free = (h%32)*128 + w.
Each partition row is one contiguous 8192-byte DRAM run -> every DMA
descriptor is a power-of-2 8 KiB (measured: 8/16 KiB descriptors run
at line rate +8 ns; odd sizes like 6.3/7.1 KiB pay +30-80 ns each, and
non-multiple-of-8 row counts fall off the 16-engine descriptor
balancer entirely).

Engine schedule per slice (balanced so neither ACT nor DVE paces the
post-load tail far above the store-DMA rate):
  ACT: activation-Copy of xt[:, :A] into a dead PSUM scratch with
       accum_out -> fp32 column sums (the PSUM write is free bandwidth;
       only Matmul/Memset may write bf16 to PSUM, so scratch is f32)
  DVE: tensor_reduce(add) of xt[:, A:] -> fp32 column sums
  PE : two accumulated fp32 matmuls against a constant 128x128 matrix
       of 1/2^19 -> cross-partition sum + broadcast of the mean to all
       partitions in one PSUM [128,1]
  ACT: tiny copy of the mean PSUM->SBUF
  DVE: single tensor_scalar multiply (bf16, ~4 elem/cyc/lane)
  DMA: loads on the SP HWDGE ring, stores on the ACT ring
"""
import numpy as np
import ml_dtypes

import concourse.bacc as bacc
import concourse.tile as tile
import concourse.mybir as mybir
from concourse.bass_utils import run_bass_kernel_spmd

BF16 = ml_dtypes.bfloat16

B, C, D, H, W = 8, 32, 32, 128, 128
HG, HL = 4, 32          # H split: partition dim = C*HG = 128
P = C * HG              # 128 partitions
F = HL * W              # 4096 free elements per partition
N_RED = C * H * W       # 524288 = 2**19 elements reduced per (b, d)
RECIP = 1.0 / N_RED     # exact in fp32
A_SPLIT = 2432          # ACT reduces xt[:, :A], DVE reduces xt[:, A:]

_NC = None


def _build_nc(xin_bufs=12, out_bufs=4):
    nc = bacc.Bacc("TRN2", target_bir_lowering=False, debug=False)
    x5 = nc.dram_tensor("x", [C, D, HG, HL, W], mybir.dt.bfloat16, kind="ExternalInput")
    o5 = nc.dram_tensor("out", [C, D, HG, HL, W], mybir.dt.bfloat16, kind="ExternalOutput")
    with tile.TileContext(nc) as tc:
        with (
            tc.tile_pool(name="xin", bufs=xin_bufs) as xpool,
            tc.tile_pool(name="oout", bufs=out_bufs) as opool,
            tc.tile_pool(name="small", bufs=6) as spool,
            tc.tile_pool(name="psum", bufs=2, space="PSUM") as ppool,
            tc.tile_pool(name="psc", bufs=1, space="PSUM") as scpool,
            tc.tile_pool(name="const", bufs=1) as cpool,
        ):
            recip = cpool.tile([P, P], mybir.dt.float32)
            nc.gpsimd.memset(recip[:], RECIP)
            for d in range(D):
                xt = xpool.tile([P, F], mybir.dt.bfloat16, tag="xt")
                nc.sync.dma_start(xt[:], x5[:, d])
                csa = spool.tile([P, 1], mybir.dt.float32, tag="csa")
                csb = spool.tile([P, 1], mybir.dt.float32, tag="csb")
                scratch = scpool.tile([P, A_SPLIT], mybir.dt.float32, tag="sc")
                nc.scalar.activation(
                    scratch[:], xt[:, :A_SPLIT],
                    mybir.ActivationFunctionType.Copy, accum_out=csa[:],
                )
                nc.vector.tensor_reduce(
                    csb[:], xt[:, A_SPLIT:],
                    mybir.AxisListType.X, mybir.AluOpType.add,
                )
                dv = ppool.tile([P, 1], mybir.dt.float32, tag="dv")
                nc.tensor.matmul(dv[:], recip[:], csa[:], start=True, stop=False)
                nc.tensor.matmul(dv[:], recip[:], csb[:], start=False, stop=True)
                dvs = spool.tile([P, 1], mybir.dt.float32, tag="dvs")
                nc.scalar.copy(dvs[:], dv[:])
                ot = opool.tile([P, F], mybir.dt.bfloat16, tag="ot")
                nc.vector.tensor_scalar_mul(ot[:], xt[:], dvs[:])
                nc.scalar.dma_start(o5[:, d], ot[:])
    nc.compile()
    return nc


def _get_nc():
    global _NC
    if _NC is None:
        _NC = _build_nc()
    return _NC


def run(x: np.ndarray, trace: bool = False, tmpdir: str | None = None):
    """Run on 8 NeuronCores; returns (out, BassKernelResults)."""
    x = np.asarray(x)
    assert x.shape == (B, C, D, H, W), x.shape
    xb = x.astype(BF16)
    nc = _get_nc()
    in_maps = [
        {"x": np.ascontiguousarray(xb[b]).reshape(C, D, HG, HL, W)} for b in range(B)
    ]
    res = run_bass_kernel_spmd(
        nc, in_maps, core_ids=list(range(B)), trace=trace, tmpdir=tmpdir
    )
    out = np.stack(
        [r["out"].astype(np.float32).reshape(C, D, H, W) for r in res.results]
    )
    return out, res


def kernel(x: np.ndarray) -> np.ndarray:
    out, _ = run(x)
    return out
